# revision 43
# baseline (speedup 1.0000x reference)
"""Distributed Trainium2 Bass kernel for the reference attention block.

Shapes: x[2, 2048, 1024], 16 heads x 64 dim, RoPE, additive mask, softmax,
out_proj.  Sharding over 8 NeuronCores: core c = (batch b = c // 4,
head-group hg = c % 4 of 4 heads).  Per core: QKV projection for its 4 heads
(column-parallel), RoPE, attention, partial out_proj (row-parallel), then
chunked ReduceScatter(add) over the 4 cores of the same batch.

Two device graphs:
  * fast path (plain causal mask + overflow-safe scores, checked on host):
    no max-subtraction pass, q/k packed two-heads-per-[128,T]-tile, scores
    computed as S^T per 512-query block with diagonal blocks narrowed to
    valid queries, causal masking applied ON the tensor engine by
    accumulating eye^T @ (-1e9 * lower_tri) into the diagonal 128x128
    psum block, exp on ACT, context matmul with an augmented ones column
    accumulating the softmax denominator.  The PE instruction stream is
    software-pipelined (scores run LOOKAHEAD blocks ahead of context
    matmuls) and the per-head normalization (vector-engine reciprocal +
    rank-1 broadcast matmul) is deferred into the next head's stream so
    the tensor engine never waits on it.  out_proj for block ib is emitted
    under pass B of block ib+1; ReduceScatter runs in 8 [256,1024] chunks
    with a tiny warm-up collective to absorb first-collective latency.
  * general fallback (arbitrary masks / large scores): the original
    two-pass flash-style graph with host-packed mask tiles.
"""

import sys

for _p in ("/opt/trn_rl_repo",):
    if _p not in sys.path:
        sys.path.insert(0, _p)

import numpy as np
import ml_dtypes

import concourse.bass as bass
import concourse.mybir as mybir
import concourse.tile as tile
from concourse import bacc
from concourse.bass_utils import run_bass_kernel_spmd
from concourse.masks import make_identity

B, T, C = 2, 2048, 1024
H, D = 16, 64
NCORES = 8
GROUPS = [[0, 1, 2, 3], [4, 5, 6, 7]]
HPC = 4                  # heads per core
FPC = HPC * D            # 256 projected features per core (per q/k/v)
NT = T // 128            # 16 row tiles
NIB = T // 512           # 4 query blocks in pass B
BF16 = mybir.dt.bfloat16
F32 = mybir.dt.float32
NPBF16 = ml_dtypes.bfloat16

SKIP, FREE, MASKED = 0, 1, 2


def _analyze_mask(attn_mask):
    """Merged 128x128 block flags across both batches (one SPMD graph)."""
    tiles = attn_mask.reshape(B, NT, 128, NT, 128)
    skip = (tiles <= -1e8).all(axis=(2, 4))     # [B, NT, NT]
    free = (tiles == 0).all(axis=(2, 4))
    flags = np.full((NT, NT), MASKED, dtype=np.int8)
    flags[free.all(axis=0)] = FREE
    flags[skip.all(axis=0)] = SKIP
    for it in range(NT):                        # fully-masked query rows:
        if (flags[it] == SKIP).all():           # compute them masked so the
            flags[it] = MASKED                  # softmax matches the reference
    return flags


def _plan(flags):
    """Static loop structure shared by every core.

    passA[it] = runs (j0_tile, n_tiles, [masked_offsets]); each run is a
    contiguous stretch of <=4 non-SKIP key tiles.
    passB[ib] = list of (jt, needs_mask) for the 512-wide query block ib.
    """
    passA = []
    for it in range(NT):
        runs = []
        jt = 0
        while jt < NT:
            if flags[it, jt] == SKIP:
                jt += 1
                continue
            j0 = jt
            while jt < NT and jt - j0 < 4 and flags[it, jt] != SKIP:
                jt += 1
            masked = [k - j0 for k in range(j0, jt) if flags[it, k] == MASKED]
            runs.append((j0, jt - j0, masked))
        passA.append(runs)

    passB = []
    for ib in range(NIB):
        sub = flags[ib * 4:(ib + 1) * 4]        # [4, NT]
        blocks = []
        for jt in range(NT):
            col = sub[:, jt]
            if (col == SKIP).all():
                continue
            blocks.append((jt, not (col == FREE).all()))
        passB.append(blocks)
    return passA, passB


def _build_graph(flags, mfree=False, debug=False):
    passA, passB = _plan(flags)
    nA = sum(len(m) for runs in passA for (_, _, m) in runs)
    nB = sum(1 for blocks in passB for (_, msk) in blocks if msk)

    nc = bacc.Bacc(num_devices=NCORES)

    # ---- parameters (per-core shards, prepared on host) ----
    p_xT = nc.declare_dram_parameter("xT", [C, T], BF16, isOutput=False)
    p_wqkT = nc.declare_dram_parameter("wqkT", [C, 2 * FPC], BF16, isOutput=False)
    p_wvT = nc.declare_dram_parameter("wvT", [C, FPC], BF16, isOutput=False)
    p_qkb = nc.declare_dram_parameter("qkb", [1, 2 * FPC], BF16, isOutput=False)
    p_vb = nc.declare_dram_parameter("vb", [1, FPC], BF16, isOutput=False)
    p_ct = nc.declare_dram_parameter("ct", [128, T], BF16, isOutput=False)
    p_st = nc.declare_dram_parameter("st", [128, T], BF16, isOutput=False)
    p_w0 = nc.declare_dram_parameter("wout0", [128, C], BF16, isOutput=False)
    p_w1 = nc.declare_dram_parameter("wout1", [128, C], BF16, isOutput=False)
    p_ob = nc.declare_dram_parameter("obias", [1, C], BF16, isOutput=False)
    p_mA = nc.declare_dram_parameter("maskA", [max(nA, 1), 128, 128], F32,
                                     isOutput=False)
    p_mB = nc.declare_dram_parameter("maskB", [max(nB, 1), 128, 512], F32,
                                     isOutput=False)
    p_out = nc.declare_dram_parameter("out", [T // 4, C], BF16, isOutput=True)
    if debug:
        p_dqa = nc.declare_dram_parameter("dqa", [HPC, 65, T], BF16, isOutput=True)
        p_dka = nc.declare_dram_parameter("dka", [HPC, 65, T], BF16, isOutput=True)
        p_dmall = nc.declare_dram_parameter("dmall", [128, HPC * NT], F32,
                                            isOutput=True)
        p_dot = nc.declare_dram_parameter("dot", [2, 128, T], BF16, isOutput=True)
        p_dva = nc.declare_dram_parameter("dva", [NT, 128, HPC * 65], BF16,
                                          isOutput=True)


    with tile.TileContext(nc) as tc, \
            tc.tile_pool(name="static", bufs=1) as st_pool, \
            tc.tile_pool(name="sdram", bufs=1, space="DRAM") as dr_pool:
        def _t(shape, dtype, name, **k):
            return st_pool.tile(shape, dtype, name=name, tag=name, **k)

        # ---- static SBUF tensors ----
        xT = [_t([128, T], BF16, name=f"xT{i}") for i in range(8)]
        wqk = [_t([128, 2 * FPC], BF16, name=f"wqk{i}") for i in range(8)]
        wv = [_t([128, FPC], BF16, name=f"wv{i}") for i in range(8)]
        qkb = _t([1, 2 * FPC], BF16, name="qkb")
        vb = _t([1, FPC], BF16, name="vb")
        ct = _t([128, T], BF16, name="ct")
        st = _t([128, T], BF16, name="st")
        w0 = _t([128, C], BF16, name="w0")
        w1 = _t([128, C], BF16, name="w1")
        obias = _t([1, C], BF16, name="obias")
        # Q/K augmented: rows 0..63 = RoPE'd head dims, row 64 = -m (Q), 1s (K)
        qa = [_t([65, T], BF16, name=f"qa{h}") for h in range(HPC)]
        ka = [_t([65, T], BF16, name=f"ka{h}") for h in range(HPC)]
        # V augmented per key tile: [128, 4 heads x (64 dims + ones col)]
        va = [_t([128, HPC * 65], BF16, name=f"va{j}") for j in range(NT)]
        # context output, [dv, t] layout, two 128-row chunks
        ot = [_t([128, T], BF16, name=f"ot{i}") for i in range(2)]
        mall = _t([128, HPC * NT], F32, name="mall")   # running row maxes
        ident = _t([128, 128], F32, name="ident")
        ones65 = _t([65, 64], F32, name="ones65")
        ones_t = _t([1, 512], BF16, name="ones_t")

        make_identity(nc, ident[:, :])
        nc.gpsimd.memset(ones65[:, :], 1.0)
        nc.vector.memset(ones_t[:, :], 1.0)

        for i in range(8):
            nc.sync.dma_start(out=xT[i][:, :], in_=p_xT[i * 128:(i + 1) * 128, :])
            nc.sync.dma_start(out=wqk[i][:, :], in_=p_wqkT[i * 128:(i + 1) * 128, :])
            nc.sync.dma_start(out=wv[i][:, :], in_=p_wvT[i * 128:(i + 1) * 128, :])
        nc.sync.dma_start(out=qkb[:, :], in_=p_qkb[:, :])
        nc.sync.dma_start(out=vb[:, :], in_=p_vb[:, :])
        for sb, pp in ((ct, p_ct), (st, p_st),
                       (w0, p_w0), (w1, p_w1), (obias, p_ob)):
            nc.sync.dma_start(out=sb[:, :], in_=pp[:, :])

        with (
            tc.tile_pool(name="ps_big", bufs=3, space="PSUM") as ps_big,
            tc.tile_pool(name="ps_ot", bufs=2, space="PSUM") as ps_ot,
            tc.tile_pool(name="ps_rep", bufs=2, space="PSUM") as ps_rep,
            tc.tile_pool(name="sb_raw", bufs=2) as sb_raw,
            tc.tile_pool(name="sb_tmp", bufs=2) as sb_tmp,
            tc.tile_pool(name="sb_et", bufs=4) as sb_et,
            tc.tile_pool(name="sb_msk", bufs=3) as sb_msk,
            tc.tile_pool(name="sb_st", bufs=4) as sb_st,
        ):
            # ================= QKV projection + RoPE =================
            # q/k: psum[f, t] = wqk^T x (+bias); f = 2 heads per M-tile.
            for mt in range(4):            # 0,1: q heads 01/23; 2,3: k heads
                raw = sb_raw.tile([128, T], BF16, tag="raw")
                for tb in range(4):
                    ps = ps_big.tile([128, 512], F32, tag="big")
                    for kc in range(8):
                        nc.tensor.matmul(
                            ps[:, :], wqk[kc][:, mt * 128:(mt + 1) * 128],
                            xT[kc][:, tb * 512:(tb + 1) * 512],
                            start=(kc == 0), stop=False)
                    nc.tensor.matmul(
                        ps[:, :], qkb[:, mt * 128:(mt + 1) * 128],
                        ones_t[:, :], start=False, stop=True)
                    nc.scalar.copy(raw[:, tb * 512:(tb + 1) * 512], ps[:, :])
                tgt = qa if mt < 2 else ka
                rawrot = sb_raw.tile([128, T], BF16, tag="rawrot", bufs=1)
                for s in range(2):
                    r = s * 64
                    nc.sync.dma_start(out=rawrot[r:r + 32, :],
                                      in_=raw[r + 32:r + 64, :])
                    nc.sync.dma_start(out=rawrot[r + 32:r + 64, :],
                                      in_=raw[r:r + 32, :])
                tmpA = sb_tmp.tile([128, T], BF16, tag="tmpA", bufs=1)
                qk2 = sb_tmp.tile([128, T], BF16, tag="qk2", bufs=1)
                nc.vector.tensor_mul(tmpA[:, :], raw[:, :], ct[:, :])
                nc.vector.tensor_mul(qk2[:, :], rawrot[:, :], st[:, :])
                nc.vector.tensor_add(qk2[:, :], tmpA[:, :], qk2[:, :])
                for s in range(2):
                    h = (mt % 2) * 2 + s
                    r = s * 64
                    nc.sync.dma_start(out=tgt[h][0:64, :], in_=qk2[r:r + 64, :])
            for h in range(HPC):           # K ones row
                nc.gpsimd.memset(ka[h][64:65, :], 1.0)

            # v: psum[t, dv] = x^T wv (+bias), packed into va with ones cols.
            for tt in range(NT):
                ps = ps_big.tile([128, 512], F32, tag="big")
                for kc in range(8):
                    nc.tensor.matmul(
                        ps[:, 0:FPC], xT[kc][:, tt * 128:(tt + 1) * 128],
                        wv[kc][:, :], start=(kc == 0), stop=False)
                nc.tensor.matmul(ps[:, 0:FPC], ones_t[:1, 0:128], vb[:, :],
                                 start=False, stop=True)
                nc.vector.memset(va[tt][:, :], 1.0)
                nc.scalar.copy(
                    va[tt][:, :].rearrange("p (h e) -> p h e", e=65)[:, :, 0:64],
                    ps[:, 0:FPC].rearrange("p (h d) -> p h d", d=64))

            # ================= pass A (all heads): row maxes =================
            # (skipped when the host-computed Cauchy-Schwarz score bound
            #  shows exp() cannot overflow/underflow: qa row 64 stays 0)
            ia = 0
            for h in range(HPC if not mfree else 0):
                for it in range(NT):
                    col = h * NT + it
                    first = True
                    for (j0, njt, masked) in passA[it]:
                        ln = njt * 128
                        ps = ps_big.tile([128, 512], F32, tag="big")
                        nc.tensor.matmul(
                            ps[:, :ln], qa[h][0:64, it * 128:(it + 1) * 128],
                            ka[h][0:64, j0 * 128:j0 * 128 + ln],
                            start=True, stop=True)
                        for off in masked:
                            msk = sb_msk.tile([128, 128], F32, tag="mA")
                            nc.sync.dma_start(out=msk[:, :],
                                              in_=p_mA[ia % max(nA, 1)])
                            ia += 1
                            nc.vector.tensor_add(
                                ps[:, off * 128:(off + 1) * 128],
                                ps[:, off * 128:(off + 1) * 128], msk[:, :])
                        if first:
                            nc.vector.reduce_max(
                                mall[:, col:col + 1], ps[:, :ln],
                                axis=mybir.AxisListType.X)
                            first = False
                        else:
                            mtmp = sb_st.tile([128, 1], F32, tag="mtmp")
                            nc.vector.reduce_max(
                                mtmp[:, :], ps[:, :ln],
                                axis=mybir.AxisListType.X)
                            nc.vector.tensor_max(
                                mall[:, col:col + 1], mall[:, col:col + 1],
                                mtmp[:, :])
                # transpose this head's maxes to a row, negate into q row 64
                pmt = ps_rep.tile([64, 512], F32, tag="rep")
                nc.tensor.transpose(pmt[0:NT, 0:128],
                                    mall[:, h * NT:(h + 1) * NT], ident[:, :])
                msb = sb_st.tile([16, 128], BF16, tag="msb")
                nc.scalar.activation(msb[:, :], pmt[0:NT, 0:128],
                                     mybir.ActivationFunctionType.Copy,
                                     scale=-1.0)
                nc.sync.dma_start(out=qa[h][64:65, :], in_=msb[:, :])
            if mfree:
                for h in range(HPC):
                    nc.gpsimd.memset(qa[h][64:65, :], 0.0)

            # ======== pass B + out_proj + chunked ReduceScatter ========
            rs_in = [dr_pool.tile([512, C], BF16, name=f"rs_in{g}",
                                  tag=f"rs_in{g}") for g in range(NIB)]
            rs_out = [dr_pool.tile([128, C], BF16, name=f"rs_out{g}",
                                   tag=f"rs_out{g}") for g in range(NIB)]
            mb_idx = {}
            for _ib in range(NIB):
                for (_jt, _mf) in passB[_ib]:
                    if _mf:
                        mb_idx[(_ib, _jt)] = len(mb_idx)
            for ib in range(NIB):
                blocks = passB[ib]
                for h in range(HPC):
                    po = ps_ot.tile([65, 512], F32, tag="ot")
                    for bi, (jt, msk_flag) in enumerate(blocks):
                        ps = ps_big.tile([128, 512], F32, tag="big")
                        nc.tensor.matmul(
                            ps[:, :], ka[h][0:65, jt * 128:(jt + 1) * 128],
                            qa[h][0:65, ib * 512:(ib + 1) * 512],
                            start=True, stop=True)
                        if msk_flag:
                            mskb = sb_msk.tile([128, 512], F32, tag="mB")
                            nc.sync.dma_start(out=mskb[:, :],
                                              in_=p_mB[mb_idx[(ib, jt)]])
                            nc.vector.tensor_add(ps[:, :], ps[:, :],
                                                 mskb[:, :])
                        et = sb_et.tile([128, 512], BF16, tag="et")
                        nc.scalar.activation(et[:, :], ps[:, :],
                                             mybir.ActivationFunctionType.Exp)
                        nc.tensor.matmul(
                            po[:, :], va[jt][:, h * 65:(h + 1) * 65], et[:, :],
                            start=(bi == 0), stop=(bi == len(blocks) - 1))
                    linv65 = sb_st.tile([65, 512], F32, tag="linv65", bufs=2)
                    nc.vector.reciprocal(linv65[64:65, :], po[64:65, :])
                    prep_ps = ps_rep.tile([64, 512], F32, tag="rep")
                    nc.tensor.matmul(prep_ps[:, :], ones65[64:65, 0:64],
                                     linv65[64:65, :], start=True, stop=True)
                    prep = sb_st.tile([64, 512], F32, tag="prep", bufs=2)
                    nc.scalar.copy(prep[:, :], prep_ps[:, :])
                    if h % 2 == 0:
                        nc.vector.tensor_mul(
                            ot[h // 2][0:64, ib * 512:(ib + 1) * 512],
                            po[0:64, :], prep[:, :])
                    else:
                        otmp = sb_st.tile([64, 512], BF16, tag="otmp")
                        nc.vector.tensor_mul(otmp[:, :], po[0:64, :],
                                             prep[:, :])
                        nc.sync.dma_start(
                            out=ot[h // 2][64:128, ib * 512:(ib + 1) * 512],
                            in_=otmp[:, :])
                for lt in range(4):
                    tt = ib * 4 + lt
                    oo = sb_et.tile([128, C], BF16, tag="oo", bufs=3)
                    for ob in range(2):
                        ps = ps_big.tile([128, 512], F32, tag="big")
                        nc.tensor.matmul(
                            ps[:, :], ot[0][:, tt * 128:(tt + 1) * 128],
                            w0[:, ob * 512:(ob + 1) * 512],
                            start=True, stop=False)
                        nc.tensor.matmul(
                            ps[:, :], ot[1][:, tt * 128:(tt + 1) * 128],
                            w1[:, ob * 512:(ob + 1) * 512],
                            start=False, stop=False)
                        nc.tensor.matmul(
                            ps[:, :], ones_t[:1, 0:128],
                            obias[:, ob * 512:(ob + 1) * 512],
                            start=False, stop=True)
                        nc.scalar.copy(oo[:, ob * 512:(ob + 1) * 512],
                                       ps[:, :])
                    nc.sync.dma_start(
                        out=rs_in[ib][lt * 128:(lt + 1) * 128, :],
                        in_=oo[:, :])
                nc.gpsimd.collective_compute(
                    "ReduceScatter", mybir.AluOpType.add,
                    replica_groups=GROUPS,
                    ins=[rs_in[ib][:, :].opt()], outs=[rs_out[ib][:, :].opt()])
                nc.sync.dma_start(out=p_out[ib * 128:(ib + 1) * 128, :],
                                  in_=rs_out[ib][:, :])
            if debug:
                for h in range(HPC):
                    nc.sync.dma_start(out=p_dqa[h], in_=qa[h][:, :])
                    nc.sync.dma_start(out=p_dka[h], in_=ka[h][:, :])
                nc.sync.dma_start(out=p_dmall[:, :], in_=mall[:, :])
                for i in range(2):
                    nc.sync.dma_start(out=p_dot[i], in_=ot[i][:, :])
                for j in range(NT):
                    nc.sync.dma_start(out=p_dva[j], in_=va[j][:, :])

    nc.compile()
    return nc, passA, passB, nA, nB


def _build_graph_fast(bias_zero):
    """Optimized single-pass graph for the plain-causal, overflow-safe case.

    Differences vs the general path:
      * no pass A / no aug rows: q/k packed two-heads-per-tile [128, T].
      * diagonal score blocks narrowed to valid queries; the only masking is
        one static 128x128 causal 0/1 bf16 multiply on the exp output.
      * PE instruction stream software-pipelined (scores run ahead of
        context matmuls) so the tensor engine never stalls on the exp.
      * softmax denominator division via reciprocal_approx_fast + a rank-1
        PE broadcast; out_proj for block ib emitted after pass B of block
        ib+1 head 0 so its inputs are ready when the PE reaches it.
      * all non-input DMAs issued from the Pool queue (cheap dispatch).
    """
    nc = bacc.Bacc(num_devices=NCORES)

    p_xT = nc.declare_dram_parameter("xT", [C, T], BF16, isOutput=False)
    p_wqkT = nc.declare_dram_parameter("wqkT", [C, 2 * FPC], BF16, isOutput=False)
    p_wvT = nc.declare_dram_parameter("wvT", [C, FPC], BF16, isOutput=False)
    p_qkb = nc.declare_dram_parameter("qkb", [1, 2 * FPC], BF16, isOutput=False)
    p_vb = nc.declare_dram_parameter("vb", [1, FPC], BF16, isOutput=False)
    p_ct = nc.declare_dram_parameter("ct", [128, T], BF16, isOutput=False)
    p_st = nc.declare_dram_parameter("st", [128, T], BF16, isOutput=False)
    p_w0 = nc.declare_dram_parameter("wout0", [128, C], BF16, isOutput=False)
    p_w1 = nc.declare_dram_parameter("wout1", [128, C], BF16, isOutput=False)
    p_ob = nc.declare_dram_parameter("obias", [1, C], BF16, isOutput=False)
    p_eye = nc.declare_dram_parameter("eye", [128, 128], BF16, isOutput=False)
    p_trineg = nc.declare_dram_parameter("trineg", [128, 128], BF16,
                                         isOutput=False)
    p_out = nc.declare_dram_parameter("out", [T // 4, C], BF16, isOutput=True)

    ExpF = mybir.ActivationFunctionType.Exp
    LnF = mybir.ActivationFunctionType.Ln

    with tile.TileContext(nc) as tc, \
            tc.tile_pool(name="static", bufs=1) as st_pool, \
            tc.tile_pool(name="sdram", bufs=1, space="DRAM") as dr_pool:
        def _t(shape, dtype, name, **k):
            return st_pool.tile(shape, dtype, name=name, tag=name, **k)

        xT = [_t([128, T], BF16, name=f"xT{i}") for i in range(8)]
        wqk = [_t([128, 2 * FPC], BF16, name=f"wqk{i}") for i in range(8)]
        wv = [_t([128, FPC], BF16, name=f"wv{i}") for i in range(8)]
        qkb = _t([1, 2 * FPC], BF16, name="qkb")
        vb = _t([1, FPC], BF16, name="vb")
        ct = _t([128, T], BF16, name="ct")
        st = _t([128, T], BF16, name="st")
        w0 = _t([128, C], BF16, name="w0")
        w1 = _t([128, C], BF16, name="w1")
        obias = _t([1, C], BF16, name="obias")
        eye = _t([128, 128], BF16, name="eye")
        trineg = _t([128, 128], BF16, name="trineg")
        # q/k RoPE'd, two heads per tile: rows 0:64 head 2g, 64:128 head 2g+1
        qH = [_t([128, T], BF16, name=f"qH{g}") for g in range(2)]
        kH = [_t([128, T], BF16, name=f"kH{g}") for g in range(2)]
        # V per key tile: [128 keys, 4 heads x (64 dims + ones col)]
        va = [_t([128, HPC * 65], BF16, name=f"va{j}") for j in range(NT)]
        # normalized context, [dv, t] layout, two 128-row chunks
        ot = [_t([128, T], BF16, name=f"ot{i}") for i in range(2)]
        ones_t = _t([1, 512], BF16, name="ones_t")
        # all-ones [33, 64]: rows 0/32 are the stationaries for a head
        # pair's rank-1 broadcasts (matmul stationary bases must be 0/32/64,
        # so the pair's batched 1/l rows live at partitions 0 and 32).
        ones4 = _t([33, 64], BF16, name="ones4")

        nc.vector.memset(ones_t[:, :], 1.0)
        nc.vector.memset(ones4[:, :], 1.0)
        for j in range(NT):
            nc.vector.memset(va[j][:, :], 1.0)

        # ---- input loads: x + qk weights split across the sync and
        # scalar rings (parallel transfer), v weights + late-phase
        # tensors on the Pool ring (which later carries the collectives).
        for i in range(0, 8, 2):
            nc.sync.dma_start(out=wqk[i][:, :], in_=p_wqkT[i * 128:(i + 1) * 128, :])
            nc.sync.dma_start(out=xT[i][:, :], in_=p_xT[i * 128:(i + 1) * 128, :])
        for i in (1, 3):
            nc.scalar.dma_start(out=wqk[i][:, :], in_=p_wqkT[i * 128:(i + 1) * 128, :])
            nc.scalar.dma_start(out=xT[i][:, :], in_=p_xT[i * 128:(i + 1) * 128, :])
        nc.scalar.dma_start(out=ct[:, :], in_=p_ct[:, :])
        nc.scalar.dma_start(out=st[:, :], in_=p_st[:, :])
        for i in (5, 7):
            nc.scalar.dma_start(out=wqk[i][:, :], in_=p_wqkT[i * 128:(i + 1) * 128, :])
            nc.scalar.dma_start(out=xT[i][:, :], in_=p_xT[i * 128:(i + 1) * 128, :])
        for i in range(8):
            nc.gpsimd.dma_start(out=wv[i][:, :], in_=p_wvT[i * 128:(i + 1) * 128, :])
        nc.gpsimd.dma_start(out=eye[:, :], in_=p_eye[:, :])
        nc.gpsimd.dma_start(out=trineg[:, :], in_=p_trineg[:, :])
        nc.gpsimd.dma_start(out=w0[:, :], in_=p_w0[:, :])
        nc.gpsimd.dma_start(out=w1[:, :], in_=p_w1[:, :])
        if not bias_zero:
            nc.gpsimd.dma_start(out=qkb[:, :], in_=p_qkb[:, :])
            nc.gpsimd.dma_start(out=vb[:, :], in_=p_vb[:, :])
            nc.gpsimd.dma_start(out=obias[:, :], in_=p_ob[:, :])

        with (
            tc.tile_pool(name="psum", bufs=1, space="PSUM") as ps_pool,
            tc.tile_pool(name="work", bufs=1) as wk_pool,
        ):
            # ================= QKV projection + RoPE =================
            for mt in range(4):        # 0,1: q heads 01/23; 2,3: k heads
                raw = wk_pool.tile([128, T], BF16, tag="raw", bufs=2)
                if mt == 0:
                    # kc-outer so the PE consumes each x tile as its DMA
                    # lands instead of stalling on the full x load.
                    pss = [ps_pool.tile([128, 512], F32, tag="s", bufs=4,
                                        name=f"ps0_{tb}") for tb in range(4)]
                    for kc in range(8):
                        for tb in range(4):
                            nc.tensor.matmul(
                                pss[tb][:, :], wqk[kc][:, 0:128],
                                xT[kc][:, tb * 512:(tb + 1) * 512],
                                start=(kc == 0), stop=(kc == 7 and bias_zero))
                    for tb in range(4):
                        if not bias_zero:
                            nc.tensor.matmul(
                                pss[tb][:, :], qkb[:, 0:128],
                                ones_t[:, :], start=False, stop=True)
                        nc.scalar.copy(raw[:, tb * 512:(tb + 1) * 512],
                                       pss[tb][:, :])
                    pss = None
                else:
                    for tb in range(4):
                        ps = ps_pool.tile([128, 512], F32, tag="s", bufs=4)
                        for kc in range(8):
                            nc.tensor.matmul(
                                ps[:, :], wqk[kc][:, mt * 128:(mt + 1) * 128],
                                xT[kc][:, tb * 512:(tb + 1) * 512],
                                start=(kc == 0), stop=(kc == 7 and bias_zero))
                        if not bias_zero:
                            nc.tensor.matmul(
                                ps[:, :], qkb[:, mt * 128:(mt + 1) * 128],
                                ones_t[:, :], start=False, stop=True)
                        nc.scalar.copy(raw[:, tb * 512:(tb + 1) * 512],
                                       ps[:, :])
                rawrot = wk_pool.tile([128, T], BF16, tag="rawrot", bufs=2)
                for s in range(2):
                    r = s * 64
                    nc.gpsimd.dma_start(out=rawrot[r:r + 32, :],
                                        in_=raw[r + 32:r + 64, :])
                    nc.gpsimd.dma_start(out=rawrot[r + 32:r + 64, :],
                                        in_=raw[r:r + 32, :])
                tgt = (qH if mt < 2 else kH)[mt % 2]
                tmpA = wk_pool.tile([128, T], BF16, tag="tmpA", bufs=2)
                tmpB = wk_pool.tile([128, T], BF16, tag="tmpB", bufs=2)
                nc.vector.tensor_mul(tmpA[:, :], raw[:, :], ct[:, :])
                nc.vector.tensor_mul(tmpB[:, :], rawrot[:, :], st[:, :])
                nc.vector.tensor_add(tgt[:, :], tmpA[:, :], tmpB[:, :])

            # v: psum[t, dv] = x^T wv (+bias), packed into va (ones cols
            # pre-set by the memsets above).
            for tt in range(NT):
                ps = ps_pool.tile([128, 512], F32, tag="s", bufs=4)
                for kc in range(8):
                    nc.tensor.matmul(
                        ps[:, 0:FPC], xT[kc][:, tt * 128:(tt + 1) * 128],
                        wv[kc][:, :], start=(kc == 0),
                        stop=(kc == 7 and bias_zero))
                if not bias_zero:
                    nc.tensor.matmul(ps[:, 0:FPC], ones_t[:1, 0:128], vb[:, :],
                                     start=False, stop=True)
                nc.scalar.copy(
                    va[tt][:, :].rearrange("p (h e) -> p h e", e=65)[:, :, 0:64],
                    ps[:, 0:FPC].rearrange("p (h d) -> p h d", d=64))

            # ======== pass B + out_proj + chunked ReduceScatter ========
            # 8 ReduceScatter chunks of [256, C]: core r of a group owns
            # out positions jb*256 + 64r .. +64 for each chunk jb.
            NJB = 2 * NIB
            rs_in = [dr_pool.tile([256, C], BF16, name=f"rs_in{g}",
                                  tag=f"rs_in{g}") for g in range(NJB)]
            rs_out = [dr_pool.tile([64, C], BF16, name=f"rs_out{g}",
                                   tag=f"rs_out{g}") for g in range(NJB)]

            # tiny warm-up collective: pays the first-collective ramp/sync
            # cost while the PE is busy with QKV.
            warm_in = dr_pool.tile([4, 64], BF16, name="warm_in",
                                   tag="warm_in")
            warm_out = dr_pool.tile([1, 64], BF16, name="warm_out",
                                    tag="warm_out")
            wz = wk_pool.tile([4, 64], BF16, tag="wz", bufs=1)
            nc.vector.memset(wz[:, :], 0.0)
            nc.gpsimd.dma_start(out=warm_in[:, :], in_=wz[:, :])
            nc.gpsimd.collective_compute(
                "ReduceScatter", mybir.AluOpType.add, replica_groups=GROUPS,
                ins=[warm_in[:, :].opt()], outs=[warm_out[:, :].opt()])

            def emit_outproj(ib):
                for lt in range(4):
                    tt = ib * 4 + lt
                    jb, half = divmod(tt, 2)
                    oo = wk_pool.tile([128, C], BF16, tag="oo", bufs=3)
                    for ob in range(2):
                        ps = ps_pool.tile([128, 512], F32, tag="op", bufs=2)
                        nc.tensor.matmul(
                            ps[:, :], ot[0][:, tt * 128:(tt + 1) * 128],
                            w0[:, ob * 512:(ob + 1) * 512],
                            start=True, stop=False)
                        nc.tensor.matmul(
                            ps[:, :], ot[1][:, tt * 128:(tt + 1) * 128],
                            w1[:, ob * 512:(ob + 1) * 512],
                            start=False, stop=bias_zero)
                        if not bias_zero:
                            nc.tensor.matmul(
                                ps[:, :], ones_t[:1, 0:128],
                                obias[:, ob * 512:(ob + 1) * 512],
                                start=False, stop=True)
                        nc.vector.tensor_scalar_add(
                            oo[:, ob * 512:(ob + 1) * 512], ps[:, :], 0.0)
                    nc.sync.dma_start(
                        out=rs_in[jb][half * 128:(half + 1) * 128, :],
                        in_=oo[:, :])
                    if half == 1:
                        nc.gpsimd.collective_compute(
                            "ReduceScatter", mybir.AluOpType.add,
                            replica_groups=GROUPS,
                            ins=[rs_in[jb][:, :].opt()],
                            outs=[rs_out[jb][:, :].opt()])
                        nc.gpsimd.dma_start(
                            out=p_out[jb * 64:(jb + 1) * 64, :],
                            in_=rs_out[jb][:, :])

            LOOKAHEAD = 3
            pending_divs = []             # closures finishing previous block
            nums = []                     # numerator tiles of current block

            def flush_div(n=1):
                for _ in range(min(n, len(pending_divs))):
                    pending_divs.pop(0)()

            for ib in range(NIB):
                for h in range(HPC):
                    g, r0 = h // 2, 64 * (h % 2)
                    blocks = [(jt, 0, 512) for jt in range(4 * ib)]
                    blocks += [(4 * ib + lt, 128 * lt, 512 - 128 * lt)
                               for lt in range(4)]
                    nb = len(blocks)
                    po = ps_pool.tile([65, 512], F32, tag="po", bufs=2)
                    ets = [None] * nb
                    for k in range(nb + LOOKAHEAD):
                        if k < nb:
                            jt, q0, W = blocks[k]
                            diag = jt >= 4 * ib
                            ps = ps_pool.tile([128, 512], F32, tag="s", bufs=4)
                            nc.tensor.matmul(
                                ps[:, 0:W],
                                kH[g][r0:r0 + 64, jt * 128:(jt + 1) * 128],
                                qH[g][r0:r0 + 64,
                                      ib * 512 + q0:(ib + 1) * 512],
                                start=True, stop=not diag,
                                skip_group_check=True)
                            if diag:
                                # causal mask on the PE: the 128x128 block
                                # at the diagonal gets -1e9 above it via
                                # eye^T @ trineg accumulated into the psum.
                                nc.tensor.matmul(
                                    ps[:, 0:128], eye[:, :], trineg[:, :],
                                    start=False, stop=True,
                                    skip_group_check=True)
                            et = wk_pool.tile([128, 512], BF16, tag="et",
                                              bufs=LOOKAHEAD + 2)
                            nc.scalar.activation(et[:, 0:W], ps[:, 0:W], ExpF)
                            ets[k] = (et, jt, q0, W)
                        if h in (0, 3) and k in (4, 6):
                            flush_div()    # previous pair's normalization
                        kk = k - LOOKAHEAD
                        if 0 <= kk < nb:
                            et, jt, q0, W = ets[kk]
                            nc.tensor.matmul(
                                po[:, q0:512],
                                va[jt][:, h * 65:(h + 1) * 65], et[:, 0:W],
                                start=(kk == 0), stop=(kk == nb - 1),
                                skip_group_check=True)
                            ets[kk] = None
                    # evict the context numerator (frees the po bank) and
                    # stage the denominator row into the per-ib batch tile
                    # via an SBUF-SBUF DMA (the only partition remapper).
                    num = wk_pool.tile([64, 512], BF16, tag="num", bufs=6)
                    nc.scalar.copy(num[:, :], po[0:64, :])
                    lev = wk_pool.tile([65, 512], F32, tag="lev", bufs=2)
                    nc.vector.tensor_scalar_add(lev[64:65, :],
                                                po[64:65, :], 0.0)
                    if h % 2 == 0:
                        ldenoms = wk_pool.tile([33, 512], F32, tag="lden",
                                               bufs=2)
                        nc.vector.memset(ldenoms[:, :], 1.0)
                    nc.sync.dma_start(out=ldenoms[32 * (h % 2):
                                                  32 * (h % 2) + 1, :],
                                      in_=lev[64:65, :])
                    nums.append(num)
                    if h % 2 == 1:
                        # one batched reciprocal per head pair; the rank-1
                        # broadcasts + scales are deferred into a later PE
                        # stream (the PE never waits on 1/l).
                        linv4 = wk_pool.tile([33, 512], F32, tag="linv4",
                                             bufs=2)
                        nc.vector.reciprocal(linv4[:, :], ldenoms[:, :])
                        linv4b = wk_pool.tile([33, 512], BF16, tag="linv4b",
                                              bufs=2)
                        nc.vector.tensor_scalar_add(linv4b[:, :],
                                                    linv4[:, :], 0.0)
                        for hh in (h - 1, h):
                            def _div(num=nums[hh % 2], linv4b=linv4b,
                                     g=hh // 2, ib=ib, odd=(hh % 2 == 1),
                                     hh=hh):
                                prep_ps = ps_pool.tile([64, 512], F32,
                                                       tag="s", bufs=4)
                                nc.tensor.matmul(
                                    prep_ps[:, :],
                                    ones4[32 * (hh % 2):32 * (hh % 2) + 1, :],
                                    linv4b[32 * (hh % 2):32 * (hh % 2) + 1, :],
                                    start=True, stop=True)
                                prep = wk_pool.tile([64, 512], BF16,
                                                    tag="prep", bufs=2)
                                nc.vector.tensor_scalar_add(
                                    prep[:, :], prep_ps[:, :], 0.0)
                                if not odd:
                                    nc.vector.tensor_mul(
                                        ot[g][0:64,
                                              ib * 512:(ib + 1) * 512],
                                        num[:, :], prep[:, :])
                                else:
                                    otmp = wk_pool.tile([64, 512], BF16,
                                                        tag="otmp", bufs=2)
                                    nc.vector.tensor_mul(
                                        otmp[:, :], num[:, :], prep[:, :])
                                    nc.sync.dma_start(
                                        out=ot[g][64:128,
                                                  ib * 512:(ib + 1) * 512],
                                        in_=otmp[:, :])
                            pending_divs.append(_div)
                        nums = []
                    if ib > 0 and h == 0:
                        emit_outproj(ib - 1)
            flush_div(4)
            emit_outproj(NIB - 1)

    nc.compile()
    return nc


# ---------------------------------------------------------------------------
# host side
# ---------------------------------------------------------------------------

_GRAPH_CACHE = {}


def _rope_tables(cos, sin):
    cosT = np.ascontiguousarray(cos.T.astype(np.float32))    # [64, T]
    sinT = np.ascontiguousarray(sin.T.astype(np.float32))
    sin_r = np.concatenate([-sinT[0:32], sinT[32:64]], axis=0)   # rotate sign
    ct = np.tile(cosT, (2, 1))
    st = np.tile(sin_r, (2, 1))
    return ct.astype(NPBF16), st.astype(NPBF16)


def _pack_masks(attn_mask, b, passA, passB, nA, nB):
    mb = attn_mask[b, 0]                                     # [T, T] f32
    mA = np.zeros((max(nA, 1), 128, 128), dtype=np.float32)
    idx = 0
    for it in range(NT):
        for (j0, njt, masked) in passA[it]:
            for off in masked:
                jt = j0 + off
                mA[idx] = mb[it * 128:(it + 1) * 128, jt * 128:(jt + 1) * 128]
                idx += 1
    mB = np.zeros((max(nB, 1), 128, 512), dtype=np.float32)
    idx = 0
    for ib in range(NIB):
        for (jt, msk_flag) in passB[ib]:
            if msk_flag:
                mB[idx] = mb[ib * 512:(ib + 1) * 512,
                             jt * 128:(jt + 1) * 128].T
                idx += 1
    return mA, mB


def _prep_core(inputs, c, passA, passB, nA, nB, mask_cache):
    b, hg = divmod(c, 4)
    f0 = hg * FPC

    x = inputs["x"][b]                                       # [T, C]
    xT = np.ascontiguousarray(x.T).astype(NPBF16)            # [C, T]

    scale = 1.0 / np.sqrt(D)                    # folded into q weights/bias
    qw = inputs["qkv_weight"]                                # [3C, C]
    qs = qw[f0:f0 + FPC] * scale
    ks = qw[C + f0:C + f0 + FPC]
    vs = qw[2 * C + f0:2 * C + f0 + FPC]
    wqkT = np.ascontiguousarray(np.concatenate([qs, ks], 0).T).astype(NPBF16)
    wvT = np.ascontiguousarray(vs.T).astype(NPBF16)

    qb = inputs["qkv_bias"]
    qkb = np.concatenate([qb[f0:f0 + FPC] * scale,
                          qb[C + f0:C + f0 + FPC]])[None, :].astype(NPBF16)
    vb = qb[2 * C + f0:2 * C + f0 + FPC][None, :].astype(NPBF16)

    wout = inputs["out_proj_weight"]                         # [C, C]
    wsh = np.ascontiguousarray(wout[:, f0:f0 + FPC].T)       # [256, C]
    w0 = wsh[0:128].astype(NPBF16)
    w1 = wsh[128:256].astype(NPBF16)
    ob = (inputs["out_proj_bias"] if hg == 0
          else np.zeros_like(inputs["out_proj_bias"]))[None, :].astype(NPBF16)

    if b not in mask_cache:
        mask_cache[b] = _pack_masks(inputs["attn_mask"], b, passA, passB,
                                    nA, nB)
    mA, mB = mask_cache[b]

    ct, st = _rope_tables(inputs["cos"], inputs["sin"])

    return dict(xT=xT, wqkT=wqkT, wvT=wvT, qkb=qkb, vb=vb, ct=ct, st=st,
                wout0=w0, wout1=w1, obias=ob, maskA=mA, maskB=mB)


def _score_bound_safe(inputs, attn_mask):
    '''True if exp(S + mask) cannot overflow/underflow without row-max
    subtraction.  RoPE is a per-pair rotation, so L2 norms of q/k rows are
    preserved and max|S| <= max_i|q_i| * max_j|k_j| / sqrt(D) per head.'''
    if (attn_mask <= -1e8).all(axis=3).any():
        return False                      # fully-masked rows need the m path
    x = np.asarray(inputs["x"], dtype=np.float32).reshape(-1, C)
    w = np.asarray(inputs["qkv_weight"], dtype=np.float32)
    b = np.asarray(inputs["qkv_bias"], dtype=np.float32)
    q = x @ w[:C].T + b[:C]
    k = x @ w[C:2 * C].T + b[C:2 * C]
    qn = np.linalg.norm(q.reshape(-1, H, D), axis=2).max(axis=0)   # per head
    kn = np.linalg.norm(k.reshape(-1, H, D), axis=2).max(axis=0)
    bound = (qn * kn).max() / np.sqrt(D) + max(attn_mask.max(), 0.0)
    return bound < 70.0


def _is_plain_causal(attn_mask):
    """True iff the mask is exactly the standard causal pattern for both
    batches: 0 on/below the diagonal, <= -1e8 strictly above."""
    q = np.arange(T)[:, None]
    k = np.arange(T)[None, :]
    valid = q >= k
    for b in range(B):
        m = attn_mask[b, 0]
        if not (m[valid] == 0).all():
            return False
        if not (m[~valid] <= -1e8).all():
            return False
    return True


def _prep_core_fast(inputs, c, eye, trineg):
    b, hg = divmod(c, 4)
    f0 = hg * FPC

    x = inputs["x"][b]                                       # [T, C]
    xT = np.ascontiguousarray(x.T).astype(NPBF16)            # [C, T]

    scale = 1.0 / np.sqrt(D)                    # folded into q weights/bias
    qw = inputs["qkv_weight"]                                # [3C, C]
    qs = qw[f0:f0 + FPC] * scale
    ks = qw[C + f0:C + f0 + FPC]
    vs = qw[2 * C + f0:2 * C + f0 + FPC]
    wqkT = np.ascontiguousarray(np.concatenate([qs, ks], 0).T).astype(NPBF16)
    wvT = np.ascontiguousarray(vs.T).astype(NPBF16)

    qb = inputs["qkv_bias"]
    qkb = np.concatenate([qb[f0:f0 + FPC] * scale,
                          qb[C + f0:C + f0 + FPC]])[None, :].astype(NPBF16)
    vb = qb[2 * C + f0:2 * C + f0 + FPC][None, :].astype(NPBF16)

    wout = inputs["out_proj_weight"]                         # [C, C]
    wsh = np.ascontiguousarray(wout[:, f0:f0 + FPC].T)       # [256, C]
    w0 = wsh[0:128].astype(NPBF16)
    w1 = wsh[128:256].astype(NPBF16)
    ob = (inputs["out_proj_bias"] if hg == 0
          else np.zeros_like(inputs["out_proj_bias"]))[None, :].astype(NPBF16)

    ct, st = _rope_tables(inputs["cos"], inputs["sin"])

    return dict(xT=xT, wqkT=wqkT, wvT=wvT, qkb=qkb, vb=vb, ct=ct, st=st,
                wout0=w0, wout1=w1, obias=ob, eye=eye, trineg=trineg)


def _run(inputs, trace=False):
    attn_mask = np.asarray(inputs["attn_mask"], dtype=np.float32)
    mfree = _score_bound_safe(inputs, attn_mask)
    bias_zero = (not np.asarray(inputs["qkv_bias"]).any()
                 and not np.asarray(inputs["out_proj_bias"]).any())

    fast = mfree and _is_plain_causal(attn_mask)
    if fast:
        key = ("fast", bias_zero)
        if key not in _GRAPH_CACHE:
            _GRAPH_CACHE[key] = _build_graph_fast(bias_zero)
        nc = _GRAPH_CACHE[key]
        eye = np.eye(128, dtype=NPBF16)
        trineg = (-1e9 * (np.arange(128)[None, :] <
                          np.arange(128)[:, None])).astype(NPBF16)
        in_maps = [_prep_core_fast(inputs, c, eye, trineg)
                   for c in range(NCORES)]
    else:
        flags = _analyze_mask(attn_mask)
        key = (flags.tobytes(), mfree)
        if key not in _GRAPH_CACHE:
            _GRAPH_CACHE[key] = _build_graph(flags, mfree=mfree)
        nc, passA, passB, nA, nB = _GRAPH_CACHE[key]
        mask_cache = {}
        in_maps = [_prep_core(inputs, c, passA, passB, nA, nB, mask_cache)
                   for c in range(NCORES)]

    res = run_bass_kernel_spmd(nc, in_maps, list(range(NCORES)), trace=trace)
    _run.last_exec_time_ns = res.exec_time_ns

    out = np.empty((B, T, C), dtype=np.float32)
    for c in range(NCORES):
        b, r = divmod(c, 4)
        sh = np.asarray(res.results[c]["out"], dtype=np.float32)
        if fast:
            for jb in range(2 * NIB):
                out[b, jb * 256 + r * 64:jb * 256 + (r + 1) * 64, :] = \
                    sh[jb * 64:(jb + 1) * 64]
        else:
            for ib in range(NIB):
                out[b, ib * 512 + r * 128:ib * 512 + (r + 1) * 128, :] = \
                    sh[ib * 128:(ib + 1) * 128]
    return out


_run.last_exec_time_ns = None


def kernel(**inputs):
    return _run(inputs, trace=False)



# revision 45
# speedup vs baseline: 1.0694x; 1.0694x over previous
"""Distributed Trainium2 Bass kernel for the reference attention block.

Shapes: x[2, 2048, 1024], 16 heads x 64 dim, RoPE, additive mask, softmax,
out_proj.  Sharding over 8 NeuronCores: core c = (batch b = c // 4,
head-group hg = c % 4 of 4 heads).  Per core: QKV projection for its 4 heads
(column-parallel), RoPE, attention, partial out_proj (row-parallel), then
chunked ReduceScatter(add) over the 4 cores of the same batch.

Two device graphs:
  * fast path (plain causal mask + overflow-safe scores, checked on host):
    no max-subtraction pass, q/k packed two-heads-per-[128,T]-tile, scores
    computed as S^T per 512-query block with diagonal blocks narrowed to
    valid queries, causal masking applied ON the tensor engine by
    accumulating eye^T @ (-1e9 * lower_tri) into the diagonal 128x128
    psum block, exp on ACT, context matmul with an augmented ones column
    accumulating the softmax denominator.  The PE instruction stream is
    software-pipelined (scores run LOOKAHEAD blocks ahead of context
    matmuls) and the per-head normalization (vector-engine reciprocal +
    rank-1 broadcast matmul) is deferred into the next head's stream so
    the tensor engine never waits on it.  out_proj for block ib is emitted
    under pass B of block ib+1; ReduceScatter runs in 8 [256,1024] chunks
    with a tiny warm-up collective to absorb first-collective latency.
  * general fallback (arbitrary masks / large scores): the original
    two-pass flash-style graph with host-packed mask tiles.
"""

import sys

for _p in ("/opt/trn_rl_repo",):
    if _p not in sys.path:
        sys.path.insert(0, _p)

import numpy as np
import ml_dtypes

import concourse.bass as bass
import concourse.mybir as mybir
import concourse.tile as tile
from concourse import bacc
from concourse.bass_utils import run_bass_kernel_spmd
from concourse.masks import make_identity

B, T, C = 2, 2048, 1024
H, D = 16, 64
NCORES = 8
GROUPS = [[0, 1, 2, 3], [4, 5, 6, 7]]
HPC = 4                  # heads per core
FPC = HPC * D            # 256 projected features per core (per q/k/v)
NT = T // 128            # 16 row tiles
NIB = T // 512           # 4 query blocks in pass B
BF16 = mybir.dt.bfloat16
F32 = mybir.dt.float32
NPBF16 = ml_dtypes.bfloat16

SKIP, FREE, MASKED = 0, 1, 2


def _analyze_mask(attn_mask):
    """Merged 128x128 block flags across both batches (one SPMD graph)."""
    tiles = attn_mask.reshape(B, NT, 128, NT, 128)
    skip = (tiles <= -1e8).all(axis=(2, 4))     # [B, NT, NT]
    free = (tiles == 0).all(axis=(2, 4))
    flags = np.full((NT, NT), MASKED, dtype=np.int8)
    flags[free.all(axis=0)] = FREE
    flags[skip.all(axis=0)] = SKIP
    for it in range(NT):                        # fully-masked query rows:
        if (flags[it] == SKIP).all():           # compute them masked so the
            flags[it] = MASKED                  # softmax matches the reference
    return flags


def _plan(flags):
    """Static loop structure shared by every core.

    passA[it] = runs (j0_tile, n_tiles, [masked_offsets]); each run is a
    contiguous stretch of <=4 non-SKIP key tiles.
    passB[ib] = list of (jt, needs_mask) for the 512-wide query block ib.
    """
    passA = []
    for it in range(NT):
        runs = []
        jt = 0
        while jt < NT:
            if flags[it, jt] == SKIP:
                jt += 1
                continue
            j0 = jt
            while jt < NT and jt - j0 < 4 and flags[it, jt] != SKIP:
                jt += 1
            masked = [k - j0 for k in range(j0, jt) if flags[it, k] == MASKED]
            runs.append((j0, jt - j0, masked))
        passA.append(runs)

    passB = []
    for ib in range(NIB):
        sub = flags[ib * 4:(ib + 1) * 4]        # [4, NT]
        blocks = []
        for jt in range(NT):
            col = sub[:, jt]
            if (col == SKIP).all():
                continue
            blocks.append((jt, not (col == FREE).all()))
        passB.append(blocks)
    return passA, passB


def _build_graph(flags, mfree=False, debug=False):
    passA, passB = _plan(flags)
    nA = sum(len(m) for runs in passA for (_, _, m) in runs)
    nB = sum(1 for blocks in passB for (_, msk) in blocks if msk)

    nc = bacc.Bacc(num_devices=NCORES)

    # ---- parameters (per-core shards, prepared on host) ----
    p_xT = nc.declare_dram_parameter("xT", [C, T], BF16, isOutput=False)
    p_wqkT = nc.declare_dram_parameter("wqkT", [C, 2 * FPC], BF16, isOutput=False)
    p_wvT = nc.declare_dram_parameter("wvT", [C, FPC], BF16, isOutput=False)
    p_qkb = nc.declare_dram_parameter("qkb", [1, 2 * FPC], BF16, isOutput=False)
    p_vb = nc.declare_dram_parameter("vb", [1, FPC], BF16, isOutput=False)
    p_ct = nc.declare_dram_parameter("ct", [128, T], BF16, isOutput=False)
    p_st = nc.declare_dram_parameter("st", [128, T], BF16, isOutput=False)
    p_w0 = nc.declare_dram_parameter("wout0", [128, C], BF16, isOutput=False)
    p_w1 = nc.declare_dram_parameter("wout1", [128, C], BF16, isOutput=False)
    p_ob = nc.declare_dram_parameter("obias", [1, C], BF16, isOutput=False)
    p_mA = nc.declare_dram_parameter("maskA", [max(nA, 1), 128, 128], F32,
                                     isOutput=False)
    p_mB = nc.declare_dram_parameter("maskB", [max(nB, 1), 128, 512], F32,
                                     isOutput=False)
    p_out = nc.declare_dram_parameter("out", [T // 4, C], BF16, isOutput=True)
    if debug:
        p_dqa = nc.declare_dram_parameter("dqa", [HPC, 65, T], BF16, isOutput=True)
        p_dka = nc.declare_dram_parameter("dka", [HPC, 65, T], BF16, isOutput=True)
        p_dmall = nc.declare_dram_parameter("dmall", [128, HPC * NT], F32,
                                            isOutput=True)
        p_dot = nc.declare_dram_parameter("dot", [2, 128, T], BF16, isOutput=True)
        p_dva = nc.declare_dram_parameter("dva", [NT, 128, HPC * 65], BF16,
                                          isOutput=True)


    with tile.TileContext(nc) as tc, \
            tc.tile_pool(name="static", bufs=1) as st_pool, \
            tc.tile_pool(name="sdram", bufs=1, space="DRAM") as dr_pool:
        def _t(shape, dtype, name, **k):
            return st_pool.tile(shape, dtype, name=name, tag=name, **k)

        # ---- static SBUF tensors ----
        xT = [_t([128, T], BF16, name=f"xT{i}") for i in range(8)]
        wqk = [_t([128, 2 * FPC], BF16, name=f"wqk{i}") for i in range(8)]
        wv = [_t([128, FPC], BF16, name=f"wv{i}") for i in range(8)]
        qkb = _t([1, 2 * FPC], BF16, name="qkb")
        vb = _t([1, FPC], BF16, name="vb")
        ct = _t([128, T], BF16, name="ct")
        st = _t([128, T], BF16, name="st")
        w0 = _t([128, C], BF16, name="w0")
        w1 = _t([128, C], BF16, name="w1")
        obias = _t([1, C], BF16, name="obias")
        # Q/K augmented: rows 0..63 = RoPE'd head dims, row 64 = -m (Q), 1s (K)
        qa = [_t([65, T], BF16, name=f"qa{h}") for h in range(HPC)]
        ka = [_t([65, T], BF16, name=f"ka{h}") for h in range(HPC)]
        # V augmented per key tile: [128, 4 heads x (64 dims + ones col)]
        va = [_t([128, HPC * 65], BF16, name=f"va{j}") for j in range(NT)]
        # context output, [dv, t] layout, two 128-row chunks
        ot = [_t([128, T], BF16, name=f"ot{i}") for i in range(2)]
        mall = _t([128, HPC * NT], F32, name="mall")   # running row maxes
        ident = _t([128, 128], F32, name="ident")
        ones65 = _t([65, 64], F32, name="ones65")
        ones_t = _t([1, 512], BF16, name="ones_t")

        make_identity(nc, ident[:, :])
        nc.gpsimd.memset(ones65[:, :], 1.0)
        nc.vector.memset(ones_t[:, :], 1.0)

        for i in range(8):
            nc.sync.dma_start(out=xT[i][:, :], in_=p_xT[i * 128:(i + 1) * 128, :])
            nc.sync.dma_start(out=wqk[i][:, :], in_=p_wqkT[i * 128:(i + 1) * 128, :])
            nc.sync.dma_start(out=wv[i][:, :], in_=p_wvT[i * 128:(i + 1) * 128, :])
        nc.sync.dma_start(out=qkb[:, :], in_=p_qkb[:, :])
        nc.sync.dma_start(out=vb[:, :], in_=p_vb[:, :])
        for sb, pp in ((ct, p_ct), (st, p_st),
                       (w0, p_w0), (w1, p_w1), (obias, p_ob)):
            nc.sync.dma_start(out=sb[:, :], in_=pp[:, :])

        with (
            tc.tile_pool(name="ps_big", bufs=3, space="PSUM") as ps_big,
            tc.tile_pool(name="ps_ot", bufs=2, space="PSUM") as ps_ot,
            tc.tile_pool(name="ps_rep", bufs=2, space="PSUM") as ps_rep,
            tc.tile_pool(name="sb_raw", bufs=2) as sb_raw,
            tc.tile_pool(name="sb_tmp", bufs=2) as sb_tmp,
            tc.tile_pool(name="sb_et", bufs=4) as sb_et,
            tc.tile_pool(name="sb_msk", bufs=3) as sb_msk,
            tc.tile_pool(name="sb_st", bufs=4) as sb_st,
        ):
            # ================= QKV projection + RoPE =================
            # q/k: psum[f, t] = wqk^T x (+bias); f = 2 heads per M-tile.
            for mt in range(4):            # 0,1: q heads 01/23; 2,3: k heads
                raw = sb_raw.tile([128, T], BF16, tag="raw")
                for tb in range(4):
                    ps = ps_big.tile([128, 512], F32, tag="big")
                    for kc in range(8):
                        nc.tensor.matmul(
                            ps[:, :], wqk[kc][:, mt * 128:(mt + 1) * 128],
                            xT[kc][:, tb * 512:(tb + 1) * 512],
                            start=(kc == 0), stop=False)
                    nc.tensor.matmul(
                        ps[:, :], qkb[:, mt * 128:(mt + 1) * 128],
                        ones_t[:, :], start=False, stop=True)
                    nc.scalar.copy(raw[:, tb * 512:(tb + 1) * 512], ps[:, :])
                tgt = qa if mt < 2 else ka
                rawrot = sb_raw.tile([128, T], BF16, tag="rawrot", bufs=1)
                for s in range(2):
                    r = s * 64
                    nc.sync.dma_start(out=rawrot[r:r + 32, :],
                                      in_=raw[r + 32:r + 64, :])
                    nc.sync.dma_start(out=rawrot[r + 32:r + 64, :],
                                      in_=raw[r:r + 32, :])
                tmpA = sb_tmp.tile([128, T], BF16, tag="tmpA", bufs=1)
                qk2 = sb_tmp.tile([128, T], BF16, tag="qk2", bufs=1)
                nc.vector.tensor_mul(tmpA[:, :], raw[:, :], ct[:, :])
                nc.vector.tensor_mul(qk2[:, :], rawrot[:, :], st[:, :])
                nc.vector.tensor_add(qk2[:, :], tmpA[:, :], qk2[:, :])
                for s in range(2):
                    h = (mt % 2) * 2 + s
                    r = s * 64
                    nc.sync.dma_start(out=tgt[h][0:64, :], in_=qk2[r:r + 64, :])
            for h in range(HPC):           # K ones row
                nc.gpsimd.memset(ka[h][64:65, :], 1.0)

            # v: psum[t, dv] = x^T wv (+bias), packed into va with ones cols.
            for tt in range(NT):
                ps = ps_big.tile([128, 512], F32, tag="big")
                for kc in range(8):
                    nc.tensor.matmul(
                        ps[:, 0:FPC], xT[kc][:, tt * 128:(tt + 1) * 128],
                        wv[kc][:, :], start=(kc == 0), stop=False)
                nc.tensor.matmul(ps[:, 0:FPC], ones_t[:1, 0:128], vb[:, :],
                                 start=False, stop=True)
                nc.vector.memset(va[tt][:, :], 1.0)
                nc.scalar.copy(
                    va[tt][:, :].rearrange("p (h e) -> p h e", e=65)[:, :, 0:64],
                    ps[:, 0:FPC].rearrange("p (h d) -> p h d", d=64))

            # ================= pass A (all heads): row maxes =================
            # (skipped when the host-computed Cauchy-Schwarz score bound
            #  shows exp() cannot overflow/underflow: qa row 64 stays 0)
            ia = 0
            for h in range(HPC if not mfree else 0):
                for it in range(NT):
                    col = h * NT + it
                    first = True
                    for (j0, njt, masked) in passA[it]:
                        ln = njt * 128
                        ps = ps_big.tile([128, 512], F32, tag="big")
                        nc.tensor.matmul(
                            ps[:, :ln], qa[h][0:64, it * 128:(it + 1) * 128],
                            ka[h][0:64, j0 * 128:j0 * 128 + ln],
                            start=True, stop=True)
                        for off in masked:
                            msk = sb_msk.tile([128, 128], F32, tag="mA")
                            nc.sync.dma_start(out=msk[:, :],
                                              in_=p_mA[ia % max(nA, 1)])
                            ia += 1
                            nc.vector.tensor_add(
                                ps[:, off * 128:(off + 1) * 128],
                                ps[:, off * 128:(off + 1) * 128], msk[:, :])
                        if first:
                            nc.vector.reduce_max(
                                mall[:, col:col + 1], ps[:, :ln],
                                axis=mybir.AxisListType.X)
                            first = False
                        else:
                            mtmp = sb_st.tile([128, 1], F32, tag="mtmp")
                            nc.vector.reduce_max(
                                mtmp[:, :], ps[:, :ln],
                                axis=mybir.AxisListType.X)
                            nc.vector.tensor_max(
                                mall[:, col:col + 1], mall[:, col:col + 1],
                                mtmp[:, :])
                # transpose this head's maxes to a row, negate into q row 64
                pmt = ps_rep.tile([64, 512], F32, tag="rep")
                nc.tensor.transpose(pmt[0:NT, 0:128],
                                    mall[:, h * NT:(h + 1) * NT], ident[:, :])
                msb = sb_st.tile([16, 128], BF16, tag="msb")
                nc.scalar.activation(msb[:, :], pmt[0:NT, 0:128],
                                     mybir.ActivationFunctionType.Copy,
                                     scale=-1.0)
                nc.sync.dma_start(out=qa[h][64:65, :], in_=msb[:, :])
            if mfree:
                for h in range(HPC):
                    nc.gpsimd.memset(qa[h][64:65, :], 0.0)

            # ======== pass B + out_proj + chunked ReduceScatter ========
            rs_in = [dr_pool.tile([512, C], BF16, name=f"rs_in{g}",
                                  tag=f"rs_in{g}") for g in range(NIB)]
            rs_out = [dr_pool.tile([128, C], BF16, name=f"rs_out{g}",
                                   tag=f"rs_out{g}") for g in range(NIB)]
            mb_idx = {}
            for _ib in range(NIB):
                for (_jt, _mf) in passB[_ib]:
                    if _mf:
                        mb_idx[(_ib, _jt)] = len(mb_idx)
            for ib in range(NIB):
                blocks = passB[ib]
                for h in range(HPC):
                    po = ps_ot.tile([65, 512], F32, tag="ot")
                    for bi, (jt, msk_flag) in enumerate(blocks):
                        ps = ps_big.tile([128, 512], F32, tag="big")
                        nc.tensor.matmul(
                            ps[:, :], ka[h][0:65, jt * 128:(jt + 1) * 128],
                            qa[h][0:65, ib * 512:(ib + 1) * 512],
                            start=True, stop=True)
                        if msk_flag:
                            mskb = sb_msk.tile([128, 512], F32, tag="mB")
                            nc.sync.dma_start(out=mskb[:, :],
                                              in_=p_mB[mb_idx[(ib, jt)]])
                            nc.vector.tensor_add(ps[:, :], ps[:, :],
                                                 mskb[:, :])
                        et = sb_et.tile([128, 512], BF16, tag="et")
                        nc.scalar.activation(et[:, :], ps[:, :],
                                             mybir.ActivationFunctionType.Exp)
                        nc.tensor.matmul(
                            po[:, :], va[jt][:, h * 65:(h + 1) * 65], et[:, :],
                            start=(bi == 0), stop=(bi == len(blocks) - 1))
                    linv65 = sb_st.tile([65, 512], F32, tag="linv65", bufs=2)
                    nc.vector.reciprocal(linv65[64:65, :], po[64:65, :])
                    prep_ps = ps_rep.tile([64, 512], F32, tag="rep")
                    nc.tensor.matmul(prep_ps[:, :], ones65[64:65, 0:64],
                                     linv65[64:65, :], start=True, stop=True)
                    prep = sb_st.tile([64, 512], F32, tag="prep", bufs=2)
                    nc.scalar.copy(prep[:, :], prep_ps[:, :])
                    if h % 2 == 0:
                        nc.vector.tensor_mul(
                            ot[h // 2][0:64, ib * 512:(ib + 1) * 512],
                            po[0:64, :], prep[:, :])
                    else:
                        otmp = sb_st.tile([64, 512], BF16, tag="otmp")
                        nc.vector.tensor_mul(otmp[:, :], po[0:64, :],
                                             prep[:, :])
                        nc.sync.dma_start(
                            out=ot[h // 2][64:128, ib * 512:(ib + 1) * 512],
                            in_=otmp[:, :])
                for lt in range(4):
                    tt = ib * 4 + lt
                    oo = sb_et.tile([128, C], BF16, tag="oo", bufs=3)
                    for ob in range(2):
                        ps = ps_big.tile([128, 512], F32, tag="big")
                        nc.tensor.matmul(
                            ps[:, :], ot[0][:, tt * 128:(tt + 1) * 128],
                            w0[:, ob * 512:(ob + 1) * 512],
                            start=True, stop=False)
                        nc.tensor.matmul(
                            ps[:, :], ot[1][:, tt * 128:(tt + 1) * 128],
                            w1[:, ob * 512:(ob + 1) * 512],
                            start=False, stop=False)
                        nc.tensor.matmul(
                            ps[:, :], ones_t[:1, 0:128],
                            obias[:, ob * 512:(ob + 1) * 512],
                            start=False, stop=True)
                        nc.scalar.copy(oo[:, ob * 512:(ob + 1) * 512],
                                       ps[:, :])
                    nc.sync.dma_start(
                        out=rs_in[ib][lt * 128:(lt + 1) * 128, :],
                        in_=oo[:, :])
                nc.gpsimd.collective_compute(
                    "ReduceScatter", mybir.AluOpType.add,
                    replica_groups=GROUPS,
                    ins=[rs_in[ib][:, :].opt()], outs=[rs_out[ib][:, :].opt()])
                nc.sync.dma_start(out=p_out[ib * 128:(ib + 1) * 128, :],
                                  in_=rs_out[ib][:, :])
            if debug:
                for h in range(HPC):
                    nc.sync.dma_start(out=p_dqa[h], in_=qa[h][:, :])
                    nc.sync.dma_start(out=p_dka[h], in_=ka[h][:, :])
                nc.sync.dma_start(out=p_dmall[:, :], in_=mall[:, :])
                for i in range(2):
                    nc.sync.dma_start(out=p_dot[i], in_=ot[i][:, :])
                for j in range(NT):
                    nc.sync.dma_start(out=p_dva[j], in_=va[j][:, :])

    nc.compile()
    return nc, passA, passB, nA, nB


def _build_graph_fast(bias_zero):
    """Optimized single-pass graph for the plain-causal, overflow-safe case.

    Differences vs the general path:
      * no pass A / no aug rows: q/k packed two-heads-per-tile [128, T].
      * diagonal score blocks narrowed to valid queries; the only masking is
        one static 128x128 causal 0/1 bf16 multiply on the exp output.
      * PE instruction stream software-pipelined (scores run ahead of
        context matmuls) so the tensor engine never stalls on the exp.
      * softmax denominator division via reciprocal_approx_fast + a rank-1
        PE broadcast; out_proj for block ib emitted after pass B of block
        ib+1 head 0 so its inputs are ready when the PE reaches it.
      * all non-input DMAs issued from the Pool queue (cheap dispatch).
    """
    nc = bacc.Bacc(num_devices=NCORES)

    p_xT = nc.declare_dram_parameter("xT", [C, T], BF16, isOutput=False)
    p_wqkT = nc.declare_dram_parameter("wqkT", [C, 2 * FPC], BF16, isOutput=False)
    p_wvT = nc.declare_dram_parameter("wvT", [C, FPC], BF16, isOutput=False)
    p_qkb = nc.declare_dram_parameter("qkb", [1, 2 * FPC], BF16, isOutput=False)
    p_vb = nc.declare_dram_parameter("vb", [1, FPC], BF16, isOutput=False)
    p_ct = nc.declare_dram_parameter("ct", [128, T], BF16, isOutput=False)
    p_st = nc.declare_dram_parameter("st", [128, T], BF16, isOutput=False)
    p_w0 = nc.declare_dram_parameter("wout0", [128, C], BF16, isOutput=False)
    p_w1 = nc.declare_dram_parameter("wout1", [128, C], BF16, isOutput=False)
    p_ob = nc.declare_dram_parameter("obias", [1, C], BF16, isOutput=False)
    p_eye = nc.declare_dram_parameter("eye", [128, 128], BF16, isOutput=False)
    p_trineg = nc.declare_dram_parameter("trineg", [128, 128], BF16,
                                         isOutput=False)
    p_out = nc.declare_dram_parameter("out", [T // 4, C], BF16, isOutput=True)

    ExpF = mybir.ActivationFunctionType.Exp
    LnF = mybir.ActivationFunctionType.Ln

    with tile.TileContext(nc) as tc, \
            tc.tile_pool(name="static", bufs=1) as st_pool, \
            tc.tile_pool(name="sdram", bufs=1, space="DRAM") as dr_pool:
        def _t(shape, dtype, name, **k):
            return st_pool.tile(shape, dtype, name=name, tag=name, **k)

        xT = [_t([128, T], BF16, name=f"xT{i}") for i in range(8)]
        wqk = [_t([128, 2 * FPC], BF16, name=f"wqk{i}") for i in range(8)]
        wv = [_t([128, FPC], BF16, name=f"wv{i}") for i in range(8)]
        qkb = _t([1, 2 * FPC], BF16, name="qkb")
        vb = _t([1, FPC], BF16, name="vb")
        ct = _t([128, T], BF16, name="ct")
        st = _t([128, T], BF16, name="st")
        w0 = _t([128, C], BF16, name="w0")
        w1 = _t([128, C], BF16, name="w1")
        obias = _t([1, C], BF16, name="obias")
        eye = _t([128, 128], BF16, name="eye")
        trineg = _t([128, 128], BF16, name="trineg")
        # q/k RoPE'd, two heads per tile: rows 0:64 head 2g, 64:128 head 2g+1
        qH = [_t([128, T], BF16, name=f"qH{g}") for g in range(2)]
        kH = [_t([128, T], BF16, name=f"kH{g}") for g in range(2)]
        # V per key tile: [128 keys, 4 heads x (64 dims + ones col)]
        va = [_t([128, HPC * 65], BF16, name=f"va{j}") for j in range(NT)]
        # normalized context, [dv, t] layout, two 128-row chunks
        ot = [_t([128, T], BF16, name=f"ot{i}") for i in range(2)]
        ones_t = _t([1, 512], BF16, name="ones_t")
        # all-ones [33, 64]: rows 0/32 are the stationaries for a head
        # pair's rank-1 broadcasts (matmul stationary bases must be 0/32/64,
        # so the pair's batched 1/l rows live at partitions 0 and 32).
        ones4 = _t([33, 64], BF16, name="ones4")

        nc.vector.memset(ones_t[:, :], 1.0)
        nc.vector.memset(ones4[:, :], 1.0)
        for j in range(NT):
            nc.vector.memset(va[j][:, :], 1.0)

        # ---- input loads: x + qk weights split across the sync and
        # scalar rings (parallel transfer), v weights + late-phase
        # tensors on the Pool ring (which later carries the collectives).
        for i in range(0, 8, 2):
            nc.sync.dma_start(out=wqk[i][:, :], in_=p_wqkT[i * 128:(i + 1) * 128, :])
            nc.sync.dma_start(out=xT[i][:, :], in_=p_xT[i * 128:(i + 1) * 128, :])
        for i in (1, 3):
            nc.scalar.dma_start(out=wqk[i][:, :], in_=p_wqkT[i * 128:(i + 1) * 128, :])
            nc.scalar.dma_start(out=xT[i][:, :], in_=p_xT[i * 128:(i + 1) * 128, :])
        nc.scalar.dma_start(out=ct[:, :], in_=p_ct[:, :])
        nc.scalar.dma_start(out=st[:, :], in_=p_st[:, :])
        for i in (5, 7):
            nc.scalar.dma_start(out=wqk[i][:, :], in_=p_wqkT[i * 128:(i + 1) * 128, :])
            nc.scalar.dma_start(out=xT[i][:, :], in_=p_xT[i * 128:(i + 1) * 128, :])
        for i in range(8):
            nc.gpsimd.dma_start(out=wv[i][:, :], in_=p_wvT[i * 128:(i + 1) * 128, :])
        nc.gpsimd.dma_start(out=eye[:, :], in_=p_eye[:, :])
        nc.gpsimd.dma_start(out=trineg[:, :], in_=p_trineg[:, :])
        nc.gpsimd.dma_start(out=w0[:, :], in_=p_w0[:, :])
        nc.gpsimd.dma_start(out=w1[:, :], in_=p_w1[:, :])
        if not bias_zero:
            nc.gpsimd.dma_start(out=qkb[:, :], in_=p_qkb[:, :])
            nc.gpsimd.dma_start(out=vb[:, :], in_=p_vb[:, :])
            nc.gpsimd.dma_start(out=obias[:, :], in_=p_ob[:, :])

        with (
            tc.tile_pool(name="psum", bufs=1, space="PSUM") as ps_pool,
            tc.tile_pool(name="work", bufs=1) as wk_pool,
        ):
            # ================= QKV projection + RoPE =================
            for mt in range(4):        # 0,1: q heads 01/23; 2,3: k heads
                raw = wk_pool.tile([128, T], BF16, tag="raw", bufs=2)
                if mt == 0:
                    # kc-outer so the PE consumes each x tile as its DMA
                    # lands instead of stalling on the full x load.
                    pss = [ps_pool.tile([128, 512], F32, tag="s", bufs=4,
                                        name=f"ps0_{tb}") for tb in range(4)]
                    for kc in range(8):
                        for tb in range(4):
                            nc.tensor.matmul(
                                pss[tb][:, :], wqk[kc][:, 0:128],
                                xT[kc][:, tb * 512:(tb + 1) * 512],
                                start=(kc == 0), stop=(kc == 7 and bias_zero))
                    for tb in range(4):
                        if not bias_zero:
                            nc.tensor.matmul(
                                pss[tb][:, :], qkb[:, 0:128],
                                ones_t[:, :], start=False, stop=True)
                        nc.scalar.copy(raw[:, tb * 512:(tb + 1) * 512],
                                       pss[tb][:, :])
                    pss = None
                else:
                    for tb in range(4):
                        ps = ps_pool.tile([128, 512], F32, tag="s", bufs=4)
                        for kc in range(8):
                            nc.tensor.matmul(
                                ps[:, :], wqk[kc][:, mt * 128:(mt + 1) * 128],
                                xT[kc][:, tb * 512:(tb + 1) * 512],
                                start=(kc == 0), stop=(kc == 7 and bias_zero))
                        if not bias_zero:
                            nc.tensor.matmul(
                                ps[:, :], qkb[:, mt * 128:(mt + 1) * 128],
                                ones_t[:, :], start=False, stop=True)
                        nc.scalar.copy(raw[:, tb * 512:(tb + 1) * 512],
                                       ps[:, :])
                rawrot = wk_pool.tile([128, T], BF16, tag="rawrot", bufs=2)
                for s in range(2):
                    r = s * 64
                    nc.gpsimd.dma_start(out=rawrot[r:r + 32, :],
                                        in_=raw[r + 32:r + 64, :])
                    nc.gpsimd.dma_start(out=rawrot[r + 32:r + 64, :],
                                        in_=raw[r:r + 32, :])
                tgt = (qH if mt < 2 else kH)[mt % 2]
                tmpA = wk_pool.tile([128, T], BF16, tag="tmpA", bufs=2)
                tmpB = wk_pool.tile([128, T], BF16, tag="tmpB", bufs=2)
                nc.vector.tensor_mul(tmpA[:, :], raw[:, :], ct[:, :])
                nc.vector.tensor_mul(tmpB[:, :], rawrot[:, :], st[:, :])
                nc.vector.tensor_add(tgt[:, :], tmpA[:, :], tmpB[:, :])

            # v: psum[t, dv] = x^T wv (+bias), packed into va (ones cols
            # pre-set by the memsets above).
            for tt in range(NT):
                ps = ps_pool.tile([128, 512], F32, tag="s", bufs=4)
                for kc in range(8):
                    nc.tensor.matmul(
                        ps[:, 0:FPC], xT[kc][:, tt * 128:(tt + 1) * 128],
                        wv[kc][:, :], start=(kc == 0),
                        stop=(kc == 7 and bias_zero))
                if not bias_zero:
                    nc.tensor.matmul(ps[:, 0:FPC], ones_t[:1, 0:128], vb[:, :],
                                     start=False, stop=True)
                nc.scalar.copy(
                    va[tt][:, :].rearrange("p (h e) -> p h e", e=65)[:, :, 0:64],
                    ps[:, 0:FPC].rearrange("p (h d) -> p h d", d=64))

            # ======== pass B + out_proj + chunked ReduceScatter ========
            # Decreasing ReduceScatter chunks: big chunks early (hidden
            # under pass-B compute), tiny chunks at the end (short tail).
            # Chunk k covers out rows [128*s, 128*(s+n)); core r of a
            # group owns the first-quarter .. fourth-quarter split of each.
            CHUNKS = [(0, 4), (4, 4), (8, 4), (12, 2), (14, 1), (15, 1)]
            rs_in = [dr_pool.tile([128 * n, C], BF16, name=f"rs_in{s}",
                                  tag=f"rs_in{s}") for (s, n) in CHUNKS]
            rs_out = [dr_pool.tile([32 * n, C], BF16, name=f"rs_out{s}",
                                   tag=f"rs_out{s}") for (s, n) in CHUNKS]
            TT2CHUNK = {}
            for k, (s, n) in enumerate(CHUNKS):
                for tt in range(s, s + n):
                    TT2CHUNK[tt] = (k, s, n)

            # tiny warm-up collective: pays the first-collective ramp/sync
            # cost while the PE is busy with QKV.
            warm_in = dr_pool.tile([4, 64], BF16, name="warm_in",
                                   tag="warm_in")
            warm_out = dr_pool.tile([1, 64], BF16, name="warm_out",
                                    tag="warm_out")
            wz = wk_pool.tile([4, 64], BF16, tag="wz", bufs=1)
            nc.vector.memset(wz[:, :], 0.0)
            nc.gpsimd.dma_start(out=warm_in[:, :], in_=wz[:, :])
            nc.gpsimd.collective_compute(
                "ReduceScatter", mybir.AluOpType.add, replica_groups=GROUPS,
                ins=[warm_in[:, :].opt()], outs=[warm_out[:, :].opt()])

            def emit_outproj(ib):
                for lt in range(4):
                    tt = ib * 4 + lt
                    k, s, n = TT2CHUNK[tt]
                    oo = wk_pool.tile([128, C], BF16, tag="oo", bufs=3)
                    for ob in range(2):
                        ps = ps_pool.tile([128, 512], F32, tag="op", bufs=2)
                        nc.tensor.matmul(
                            ps[:, :], ot[0][:, tt * 128:(tt + 1) * 128],
                            w0[:, ob * 512:(ob + 1) * 512],
                            start=True, stop=False)
                        nc.tensor.matmul(
                            ps[:, :], ot[1][:, tt * 128:(tt + 1) * 128],
                            w1[:, ob * 512:(ob + 1) * 512],
                            start=False, stop=bias_zero)
                        if not bias_zero:
                            nc.tensor.matmul(
                                ps[:, :], ones_t[:1, 0:128],
                                obias[:, ob * 512:(ob + 1) * 512],
                                start=False, stop=True)
                        nc.vector.tensor_scalar_add(
                            oo[:, ob * 512:(ob + 1) * 512], ps[:, :], 0.0)
                    nc.sync.dma_start(
                        out=rs_in[k][(tt - s) * 128:(tt - s + 1) * 128, :],
                        in_=oo[:, :])
                    if tt == s + n - 1:
                        nc.gpsimd.collective_compute(
                            "ReduceScatter", mybir.AluOpType.add,
                            replica_groups=GROUPS,
                            ins=[rs_in[k][:, :].opt()],
                            outs=[rs_out[k][:, :].opt()])
                        nc.gpsimd.dma_start(
                            out=p_out[32 * s:32 * (s + n), :],
                            in_=rs_out[k][:, :])

            LOOKAHEAD = 3
            pending_divs = []             # closures finishing previous block
            nums = []                     # numerator tiles of current block

            def flush_div(n=1):
                for _ in range(min(n, len(pending_divs))):
                    pending_divs.pop(0)()

            for ib in range(NIB):
                for h in range(HPC):
                    g, r0 = h // 2, 64 * (h % 2)
                    blocks = [(jt, 0, 512) for jt in range(4 * ib)]
                    blocks += [(4 * ib + lt, 128 * lt, 512 - 128 * lt)
                               for lt in range(4)]
                    nb = len(blocks)
                    po = ps_pool.tile([65, 512], F32, tag="po", bufs=2)
                    ets = [None] * nb
                    for k in range(nb + LOOKAHEAD):
                        if k < nb:
                            jt, q0, W = blocks[k]
                            diag = jt >= 4 * ib
                            ps = ps_pool.tile([128, 512], F32, tag="s", bufs=4)
                            nc.tensor.matmul(
                                ps[:, 0:W],
                                kH[g][r0:r0 + 64, jt * 128:(jt + 1) * 128],
                                qH[g][r0:r0 + 64,
                                      ib * 512 + q0:(ib + 1) * 512],
                                start=True, stop=not diag,
                                skip_group_check=True)
                            if diag:
                                # causal mask on the PE: the 128x128 block
                                # at the diagonal gets -1e9 above it via
                                # eye^T @ trineg accumulated into the psum.
                                nc.tensor.matmul(
                                    ps[:, 0:128], eye[:, :], trineg[:, :],
                                    start=False, stop=True,
                                    skip_group_check=True)
                            et = wk_pool.tile([128, 512], BF16, tag="et",
                                              bufs=LOOKAHEAD + 2)
                            nc.scalar.activation(et[:, 0:W], ps[:, 0:W], ExpF)
                            ets[k] = (et, jt, q0, W)
                        if h in (0, 3) and k in (4, 6):
                            flush_div()    # previous pair's normalization
                        kk = k - LOOKAHEAD
                        if 0 <= kk < nb:
                            et, jt, q0, W = ets[kk]
                            nc.tensor.matmul(
                                po[:, q0:512],
                                va[jt][:, h * 65:(h + 1) * 65], et[:, 0:W],
                                start=(kk == 0), stop=(kk == nb - 1),
                                skip_group_check=True)
                            ets[kk] = None
                    # evict the context numerator (frees the po bank) and
                    # stage the denominator row into the per-ib batch tile
                    # via an SBUF-SBUF DMA (the only partition remapper).
                    num = wk_pool.tile([64, 512], BF16, tag="num", bufs=6)
                    nc.vector.tensor_scalar_add(num[:, :], po[0:64, :], 0.0)
                    lev = wk_pool.tile([65, 512], F32, tag="lev", bufs=2)
                    nc.vector.tensor_scalar_add(lev[64:65, :],
                                                po[64:65, :], 0.0)
                    if h % 2 == 0:
                        ldenoms = wk_pool.tile([33, 512], F32, tag="lden",
                                               bufs=2)
                        nc.vector.memset(ldenoms[:, :], 1.0)
                    nc.sync.dma_start(out=ldenoms[32 * (h % 2):
                                                  32 * (h % 2) + 1, :],
                                      in_=lev[64:65, :])
                    nums.append(num)
                    if h % 2 == 1:
                        # one batched reciprocal per head pair; the rank-1
                        # broadcasts + scales are deferred into a later PE
                        # stream (the PE never waits on 1/l).
                        linv4 = wk_pool.tile([33, 512], F32, tag="linv4",
                                             bufs=2)
                        nc.vector.reciprocal(linv4[:, :], ldenoms[:, :])
                        linv4b = wk_pool.tile([33, 512], BF16, tag="linv4b",
                                              bufs=2)
                        nc.vector.tensor_scalar_add(linv4b[:, :],
                                                    linv4[:, :], 0.0)
                        for hh in (h - 1, h):
                            def _div(num=nums[hh % 2], linv4b=linv4b,
                                     g=hh // 2, ib=ib, odd=(hh % 2 == 1),
                                     hh=hh):
                                prep_ps = ps_pool.tile([64, 512], F32,
                                                       tag="s", bufs=4)
                                nc.tensor.matmul(
                                    prep_ps[:, :],
                                    ones4[32 * (hh % 2):32 * (hh % 2) + 1, :],
                                    linv4b[32 * (hh % 2):32 * (hh % 2) + 1, :],
                                    start=True, stop=True)
                                prep = wk_pool.tile([64, 512], BF16,
                                                    tag="prep", bufs=2)
                                nc.vector.tensor_scalar_add(
                                    prep[:, :], prep_ps[:, :], 0.0)
                                if not odd:
                                    nc.vector.tensor_mul(
                                        ot[g][0:64,
                                              ib * 512:(ib + 1) * 512],
                                        num[:, :], prep[:, :])
                                else:
                                    otmp = wk_pool.tile([64, 512], BF16,
                                                        tag="otmp", bufs=2)
                                    nc.vector.tensor_mul(
                                        otmp[:, :], num[:, :], prep[:, :])
                                    nc.sync.dma_start(
                                        out=ot[g][64:128,
                                                  ib * 512:(ib + 1) * 512],
                                        in_=otmp[:, :])
                            pending_divs.append(_div)
                        nums = []
                    if ib > 0 and h == 0:
                        emit_outproj(ib - 1)
            flush_div(4)
            emit_outproj(NIB - 1)

    nc.compile()
    return nc


# ---------------------------------------------------------------------------
# host side
# ---------------------------------------------------------------------------

_GRAPH_CACHE = {}


def _rope_tables(cos, sin):
    cosT = np.ascontiguousarray(cos.T.astype(np.float32))    # [64, T]
    sinT = np.ascontiguousarray(sin.T.astype(np.float32))
    sin_r = np.concatenate([-sinT[0:32], sinT[32:64]], axis=0)   # rotate sign
    ct = np.tile(cosT, (2, 1))
    st = np.tile(sin_r, (2, 1))
    return ct.astype(NPBF16), st.astype(NPBF16)


def _pack_masks(attn_mask, b, passA, passB, nA, nB):
    mb = attn_mask[b, 0]                                     # [T, T] f32
    mA = np.zeros((max(nA, 1), 128, 128), dtype=np.float32)
    idx = 0
    for it in range(NT):
        for (j0, njt, masked) in passA[it]:
            for off in masked:
                jt = j0 + off
                mA[idx] = mb[it * 128:(it + 1) * 128, jt * 128:(jt + 1) * 128]
                idx += 1
    mB = np.zeros((max(nB, 1), 128, 512), dtype=np.float32)
    idx = 0
    for ib in range(NIB):
        for (jt, msk_flag) in passB[ib]:
            if msk_flag:
                mB[idx] = mb[ib * 512:(ib + 1) * 512,
                             jt * 128:(jt + 1) * 128].T
                idx += 1
    return mA, mB


def _prep_core(inputs, c, passA, passB, nA, nB, mask_cache):
    b, hg = divmod(c, 4)
    f0 = hg * FPC

    x = inputs["x"][b]                                       # [T, C]
    xT = np.ascontiguousarray(x.T).astype(NPBF16)            # [C, T]

    scale = 1.0 / np.sqrt(D)                    # folded into q weights/bias
    qw = inputs["qkv_weight"]                                # [3C, C]
    qs = qw[f0:f0 + FPC] * scale
    ks = qw[C + f0:C + f0 + FPC]
    vs = qw[2 * C + f0:2 * C + f0 + FPC]
    wqkT = np.ascontiguousarray(np.concatenate([qs, ks], 0).T).astype(NPBF16)
    wvT = np.ascontiguousarray(vs.T).astype(NPBF16)

    qb = inputs["qkv_bias"]
    qkb = np.concatenate([qb[f0:f0 + FPC] * scale,
                          qb[C + f0:C + f0 + FPC]])[None, :].astype(NPBF16)
    vb = qb[2 * C + f0:2 * C + f0 + FPC][None, :].astype(NPBF16)

    wout = inputs["out_proj_weight"]                         # [C, C]
    wsh = np.ascontiguousarray(wout[:, f0:f0 + FPC].T)       # [256, C]
    w0 = wsh[0:128].astype(NPBF16)
    w1 = wsh[128:256].astype(NPBF16)
    ob = (inputs["out_proj_bias"] if hg == 0
          else np.zeros_like(inputs["out_proj_bias"]))[None, :].astype(NPBF16)

    if b not in mask_cache:
        mask_cache[b] = _pack_masks(inputs["attn_mask"], b, passA, passB,
                                    nA, nB)
    mA, mB = mask_cache[b]

    ct, st = _rope_tables(inputs["cos"], inputs["sin"])

    return dict(xT=xT, wqkT=wqkT, wvT=wvT, qkb=qkb, vb=vb, ct=ct, st=st,
                wout0=w0, wout1=w1, obias=ob, maskA=mA, maskB=mB)


def _score_bound_safe(inputs, attn_mask):
    '''True if exp(S + mask) cannot overflow/underflow without row-max
    subtraction.  RoPE is a per-pair rotation, so L2 norms of q/k rows are
    preserved and max|S| <= max_i|q_i| * max_j|k_j| / sqrt(D) per head.'''
    if (attn_mask <= -1e8).all(axis=3).any():
        return False                      # fully-masked rows need the m path
    x = np.asarray(inputs["x"], dtype=np.float32).reshape(-1, C)
    w = np.asarray(inputs["qkv_weight"], dtype=np.float32)
    b = np.asarray(inputs["qkv_bias"], dtype=np.float32)
    q = x @ w[:C].T + b[:C]
    k = x @ w[C:2 * C].T + b[C:2 * C]
    qn = np.linalg.norm(q.reshape(-1, H, D), axis=2).max(axis=0)   # per head
    kn = np.linalg.norm(k.reshape(-1, H, D), axis=2).max(axis=0)
    bound = (qn * kn).max() / np.sqrt(D) + max(attn_mask.max(), 0.0)
    return bound < 70.0


def _is_plain_causal(attn_mask):
    """True iff the mask is exactly the standard causal pattern for both
    batches: 0 on/below the diagonal, <= -1e8 strictly above."""
    q = np.arange(T)[:, None]
    k = np.arange(T)[None, :]
    valid = q >= k
    for b in range(B):
        m = attn_mask[b, 0]
        if not (m[valid] == 0).all():
            return False
        if not (m[~valid] <= -1e8).all():
            return False
    return True


def _prep_core_fast(inputs, c, eye, trineg):
    b, hg = divmod(c, 4)
    f0 = hg * FPC

    x = inputs["x"][b]                                       # [T, C]
    xT = np.ascontiguousarray(x.T).astype(NPBF16)            # [C, T]

    scale = 1.0 / np.sqrt(D)                    # folded into q weights/bias
    qw = inputs["qkv_weight"]                                # [3C, C]
    qs = qw[f0:f0 + FPC] * scale
    ks = qw[C + f0:C + f0 + FPC]
    vs = qw[2 * C + f0:2 * C + f0 + FPC]
    wqkT = np.ascontiguousarray(np.concatenate([qs, ks], 0).T).astype(NPBF16)
    wvT = np.ascontiguousarray(vs.T).astype(NPBF16)

    qb = inputs["qkv_bias"]
    qkb = np.concatenate([qb[f0:f0 + FPC] * scale,
                          qb[C + f0:C + f0 + FPC]])[None, :].astype(NPBF16)
    vb = qb[2 * C + f0:2 * C + f0 + FPC][None, :].astype(NPBF16)

    wout = inputs["out_proj_weight"]                         # [C, C]
    wsh = np.ascontiguousarray(wout[:, f0:f0 + FPC].T)       # [256, C]
    w0 = wsh[0:128].astype(NPBF16)
    w1 = wsh[128:256].astype(NPBF16)
    ob = (inputs["out_proj_bias"] if hg == 0
          else np.zeros_like(inputs["out_proj_bias"]))[None, :].astype(NPBF16)

    ct, st = _rope_tables(inputs["cos"], inputs["sin"])

    return dict(xT=xT, wqkT=wqkT, wvT=wvT, qkb=qkb, vb=vb, ct=ct, st=st,
                wout0=w0, wout1=w1, obias=ob, eye=eye, trineg=trineg)


def _run(inputs, trace=False):
    attn_mask = np.asarray(inputs["attn_mask"], dtype=np.float32)
    mfree = _score_bound_safe(inputs, attn_mask)
    bias_zero = (not np.asarray(inputs["qkv_bias"]).any()
                 and not np.asarray(inputs["out_proj_bias"]).any())

    fast = mfree and _is_plain_causal(attn_mask)
    if fast:
        key = ("fast", bias_zero)
        if key not in _GRAPH_CACHE:
            _GRAPH_CACHE[key] = _build_graph_fast(bias_zero)
        nc = _GRAPH_CACHE[key]
        eye = np.eye(128, dtype=NPBF16)
        trineg = (-1e9 * (np.arange(128)[None, :] <
                          np.arange(128)[:, None])).astype(NPBF16)
        in_maps = [_prep_core_fast(inputs, c, eye, trineg)
                   for c in range(NCORES)]
    else:
        flags = _analyze_mask(attn_mask)
        key = (flags.tobytes(), mfree)
        if key not in _GRAPH_CACHE:
            _GRAPH_CACHE[key] = _build_graph(flags, mfree=mfree)
        nc, passA, passB, nA, nB = _GRAPH_CACHE[key]
        mask_cache = {}
        in_maps = [_prep_core(inputs, c, passA, passB, nA, nB, mask_cache)
                   for c in range(NCORES)]

    res = run_bass_kernel_spmd(nc, in_maps, list(range(NCORES)), trace=trace)
    _run.last_exec_time_ns = res.exec_time_ns

    out = np.empty((B, T, C), dtype=np.float32)
    for c in range(NCORES):
        b, r = divmod(c, 4)
        sh = np.asarray(res.results[c]["out"], dtype=np.float32)
        if fast:
            for (s, n) in [(0, 4), (4, 4), (8, 4), (12, 2), (14, 1), (15, 1)]:
                out[b, 128 * s + 32 * n * r:128 * s + 32 * n * (r + 1), :] = \
                    sh[32 * s:32 * (s + n)]
        else:
            for ib in range(NIB):
                out[b, ib * 512 + r * 128:ib * 512 + (r + 1) * 128, :] = \
                    sh[ib * 128:(ib + 1) * 128]
    return out


_run.last_exec_time_ns = None


def kernel(**inputs):
    return _run(inputs, trace=False)

